# revision 4
# baseline (speedup 1.0000x reference)
"""MinGRU Trainium2 kernel.

Problem: B=8, T=4096, D=512, H=512 MinGRU:
    k = x @ Wz^T + bz;  z = sigmoid(k)
    w = x @ Wh^T + bh;  h~ = g(w),  g(w) = relu(w) + 0.5 (w>=0) | sigmoid(w) (w<0)
    h_t = (1 - z_t) * h_{t-1} + z_t * h~_t,   h_{-1} = g(h_0)
(The reference computes this recurrence in log space via cumlogsumexp; in
linear space all quantities are positive and bounded, so a direct fp32 scan
is numerically stable and matches to ~1e-4.)

Sharding: data-parallel over batch, one batch row per NeuronCore (8 cores).

Per-core device layout (everything transposed so H sits on partitions and T
on the free dim, which lets the VectorE `tensor_tensor_scan` instruction run
the recurrence along T):
    xT  (D=512, T=4096)  f32  - host pre-transposed
    wzT/whT (D=512, H=512) f32 - host pre-transposed weights (lhsT layout)
    k^T/w^T tiles computed on PE in PSUM with float32r (full-rate fp32)
    a    = sigmoid(-k - bz)                      [ScalarE, bias/scale fused]
    s    = sigmoid(w + bh)                       [ScalarE]
    r1   = relu(w + bh)                          [ScalarE]
    g    = min(s, 0.5) + r1                      [VectorE scalar_tensor_tensor]
           (identity: sigmoid(min(v,0)) = min(sigmoid(v), 0.5))
    bneg = (a - 1) * g                           [VectorE scalar_tensor_tensor]
    h    = scan: state = a*state - bneg          [VectorE tensor_tensor_scan]
    hT out (H=512, T=4096) f32 -> host transposes back
"""

import os

import numpy as np

import concourse.bass as bass
import concourse.mybir as mybir
import concourse.tile as tile
from concourse import bacc
from concourse.bass_utils import run_bass_kernel_spmd

# Problem constants (hardcoded per harness contract).
B, T, D, H = 8, 4096, 512, 512
P = 128          # partitions
DB = D // P      # 4 contraction blocks
HB = H // P      # 4 output h blocks
TC = 2048        # T chunk per elementwise tile
NT = T // TC     # 2
MM_N = 512       # matmul free-dim chunk
NCC = TC // MM_N # 4 matmul column chunks per tile

F32 = mybir.dt.float32
F32R = mybir.dt.float32r

# Stash of the last run's BassKernelResults (for test harness introspection).
LAST_RESULT = None


def _build_nc():
    nc = bacc.Bacc(
        "TRN2",
        target_bir_lowering=False,
        debug=False,
        enable_asserts=False,
        num_devices=B,
    )

    xT_d = nc.dram_tensor("xT", (D, T), F32R, kind="ExternalInput")
    wzT_d = nc.dram_tensor("wzT", (D, H), F32R, kind="ExternalInput")
    whT_d = nc.dram_tensor("whT", (D, H), F32R, kind="ExternalInput")
    bzn_d = nc.dram_tensor("bzn", (HB, P, 1), F32, kind="ExternalInput")
    bh_d = nc.dram_tensor("bh", (HB, P, 1), F32, kind="ExternalInput")
    h0g_d = nc.dram_tensor("h0g", (HB, P, 1), F32, kind="ExternalInput")
    hT_d = nc.dram_tensor("hT", (H, T), F32, kind="ExternalOutput")

    AF = mybir.ActivationFunctionType
    OP = mybir.AluOpType

    from contextlib import ExitStack

    with tile.TileContext(nc) as tc, ExitStack() as ctx:
        wpool = ctx.enter_context(tc.tile_pool(name="weights", bufs=1))
        cpool = ctx.enter_context(tc.tile_pool(name="carries", bufs=1))
        xpool = ctx.enter_context(tc.tile_pool(name="xtiles", bufs=2 * DB))
        spool = ctx.enter_context(tc.tile_pool(name="work", bufs=2))
        ppool = ctx.enter_context(tc.tile_pool(name="psum", bufs=1, space="PSUM"))

        # --- One-time setup: weights, biases, initial carries ---
        wz_sb = []
        wh_sb = []
        for db in range(DB):
            wz_t = wpool.tile([P, H], F32R, name=f"wz{db}")
            nc.sync.dma_start(wz_t[:], wzT_d.ap()[db * P:(db + 1) * P, :])
            wz_sb.append(wz_t)
            wh_t = wpool.tile([P, H], F32R, name=f"wh{db}")
            nc.sync.dma_start(wh_t[:], whT_d.ap()[db * P:(db + 1) * P, :])
            wh_sb.append(wh_t)

        bzn_sb = []
        bh_sb = []
        carry = []
        for hb in range(HB):
            bzn_t = cpool.tile([P, 1], F32, name=f"bzn{hb}")
            nc.sync.dma_start(bzn_t[:], bzn_d.ap()[hb, :, :])
            bzn_sb.append(bzn_t)
            bh_t = cpool.tile([P, 1], F32, name=f"bh{hb}")
            nc.sync.dma_start(bh_t[:], bh_d.ap()[hb, :, :])
            bh_sb.append(bh_t)
            c_t = cpool.tile([P, 1], F32, name=f"carry{hb}")
            nc.sync.dma_start(c_t[:], h0g_d.ap()[hb, :, :])
            carry.append(c_t)

        # --- Main loop ---
        for tg in range(NT):
            ts0 = tg * TC
            xt = []
            for db in range(DB):
                x_t = xpool.tile([P, TC], F32R, name="xt", tag="xt")
                nc.sync.dma_start(
                    x_t[:], xT_d.ap()[db * P:(db + 1) * P, ts0:ts0 + TC]
                )
                xt.append(x_t)

            for hb in range(HB):
                hs = slice(hb * P, (hb + 1) * P)

                kp = ppool.tile([P, TC], F32, name="kp", tag="kp")
                for db in range(DB):
                    lhsT = wz_sb[db][:, hs]
                    for cc in range(NCC):
                        cs = slice(cc * MM_N, (cc + 1) * MM_N)
                        nc.tensor.matmul(
                            kp[:, cs],
                            lhsT,
                            xt[db][:, cs],
                            start=(db == 0),
                            stop=(db == DB - 1),
                        )

                a_t = spool.tile([P, TC], F32, name="a_t", tag="a")
                nc.scalar.activation(
                    a_t[:], kp[:], AF.Sigmoid, bias=bzn_sb[hb][:], scale=-1.0
                )

                wp = ppool.tile([P, TC], F32, name="wp", tag="wp")
                for db in range(DB):
                    lhsT = wh_sb[db][:, hs]
                    for cc in range(NCC):
                        cs = slice(cc * MM_N, (cc + 1) * MM_N)
                        nc.tensor.matmul(
                            wp[:, cs],
                            lhsT,
                            xt[db][:, cs],
                            start=(db == 0),
                            stop=(db == DB - 1),
                        )

                s_t = spool.tile([P, TC], F32, name="s_t", tag="s")
                nc.scalar.activation(
                    s_t[:], wp[:], AF.Sigmoid, bias=bh_sb[hb][:], scale=1.0
                )
                r_t = spool.tile([P, TC], F32, name="r_t", tag="r")
                nc.scalar.activation(
                    r_t[:], wp[:], AF.Relu, bias=bh_sb[hb][:], scale=1.0
                )

                g_t = spool.tile([P, TC], F32, name="g_t", tag="g")
                nc.vector.scalar_tensor_tensor(
                    g_t[:], s_t[:], 0.5, r_t[:], op0=OP.min, op1=OP.add
                )
                bn_t = spool.tile([P, TC], F32, name="bn_t", tag="bn")
                nc.vector.scalar_tensor_tensor(
                    bn_t[:], a_t[:], 1.0, g_t[:], op0=OP.subtract, op1=OP.mult
                )

                h_t = spool.tile([P, TC], F32, name="h_t", tag="h")
                nc.vector.tensor_tensor_scan(
                    h_t[:], a_t[:], bn_t[:], carry[hb][:],
                    op0=OP.mult, op1=OP.subtract,
                )
                nc.vector.tensor_copy(carry[hb][:], h_t[:, TC - 1:TC])

                nc.sync.dma_start(hT_d.ap()[hs, ts0:ts0 + TC], h_t[:])

    nc.compile()
    return nc


def _host_prep(x, h_0, Wz, bz, Wh, bh):
    x = np.asarray(x, dtype=np.float32)
    h_0 = np.asarray(h_0, dtype=np.float32)
    Wz = np.asarray(Wz, dtype=np.float32)
    bz = np.asarray(bz, dtype=np.float32)
    Wh = np.asarray(Wh, dtype=np.float32)
    bh = np.asarray(bh, dtype=np.float32)

    xT = np.ascontiguousarray(np.transpose(x, (0, 2, 1)))  # (B, D, T)
    wzT = np.ascontiguousarray(Wz.T)  # (D, H)
    whT = np.ascontiguousarray(Wh.T)
    bzn = np.ascontiguousarray((-bz).reshape(HB, P, 1))
    bhv = np.ascontiguousarray(bh.reshape(HB, P, 1))

    # initial carry: g(h_0) = min(sigmoid(h_0), 0.5) + relu(h_0)
    sig = 1.0 / (1.0 + np.exp(-h_0.astype(np.float64)))
    h0g = (np.minimum(sig, 0.5) + np.maximum(h_0, 0.0)).astype(np.float32)
    h0g = np.ascontiguousarray(h0g.reshape(B, HB, P, 1))

    in_maps = []
    for i in range(B):
        in_maps.append({
            "xT": xT[i],
            "wzT": wzT,
            "whT": whT,
            "bzn": bzn,
            "bh": bhv,
            "h0g": h0g[i],
        })
    return in_maps


def kernel(x, h_0, Wz, bz, Wh, bh):
    global LAST_RESULT
    in_maps = _host_prep(x, h_0, Wz, bz, Wh, bh)
    nc = _build_nc()
    res = run_bass_kernel_spmd(
        nc,
        in_maps,
        core_ids=list(range(B)),
        trace=bool(int(os.environ.get("MINGRU_TRACE", "0"))),
    )
    LAST_RESULT = res
    out = np.empty((B, T, H), dtype=np.float32)
    for i in range(B):
        out[i] = res.results[i]["hT"].T
    return out
